# revision 15
# baseline (speedup 1.0000x reference)
"""Multi-head attention Trainium2 kernel (8 NeuronCores, SPMD).

Problem: B=2, S=2048, D=1024, H=16 heads, DK=64.
reference returns (output[B,S,D], attn_weights[B,H,S,S]).

Sharding: core c -> batch b=c//4, head group g=c%4 (4 heads, 256 dims).

Design notes:
  - All activations that feed matmuls are kept with the contraction dim on
    partitions: qT/kT [256, S] ("transposed" layout, via TensorE-transposed
    X^T), v natural [S, 260] (with a ones-column appended per head).
  - Scores are computed TRANSPOSED (scoresT[k, q] tiles): lhsT = kT chunk,
    rhs = qT. exp() runs on ScalarE straight out of PSUM into the exact
    layout the attn @ v matmul wants -- no attention-matrix transposes, no
    PSUM->SBUF copybacks in the hot loop (the main win: transpose-mode
    matmuls also don't count as PE activity for the HAM clock gate, which
    kept dropping the PE to 1.2 GHz in earlier versions).
  - Softmax max-subtraction is skipped: logits here are O(5), exp is safe.
  - The softmax denominator rides for free: v has a ones-column per head,
    so PSUM row 64 of the attn @ v output IS the row-sum of exp.
  - Normalization (x 1/sum) happens on VectorE: on the [k, q] exp tiles
    (written to DRAM as attn^T; the host transposes back), and on the
    attn-out rows before the output projection.
  - Output projection is row-parallel over head dims; host sums the 4
    partial outputs per batch (b_O is fed to exactly one core per batch).
  - All matmuls run as float32r (full-rate fp32 on the PE at N>=256);
    every producer of a matmul operand writes through a float32r-bitcast
    AP because the BIR verifier requires fp32r operands to be pre-rounded.
  - PSUM accumulation chains are never interleaved with other chains on
    the PE (interleaving two chains hard-crashes the exec unit).
"""

from contextlib import ExitStack

import numpy as np

import concourse.bass as bass
import concourse.tile as tile
from concourse import bacc, mybir
from concourse.bass import ts
from concourse.bass_utils import run_bass_kernel_spmd
from concourse.masks import make_identity

B, S, D, H, DK = 2, 2048, 1024, 16, 64
HPC = 4                # heads per core
HD = HPC * DK          # 256 head dims per core
DK1 = DK + 1           # head dims + ones column
P = 128
N_CORES = 8
F32 = mybir.dt.float32
F32R = mybir.dt.float32r
AX = mybir.AxisListType.X
AFT = mybir.ActivationFunctionType


def _r(ap):
    """bitcast an fp32 AP to float32r for full-rate PE matmul"""
    return ap.bitcast(F32R)


def build_nc():
    nc = bacc.Bacc("TRN2", target_bir_lowering=False, debug=False,
                   num_devices=N_CORES)

    xq = nc.dram_tensor("xq", [S, D], F32, kind="ExternalInput").ap()
    xk = nc.dram_tensor("xk", [S, D], F32, kind="ExternalInput").ap()
    xv = nc.dram_tensor("xv", [S, D], F32, kind="ExternalInput").ap()
    wq = nc.dram_tensor("wq", [HD, D], F32, kind="ExternalInput").ap()
    wk = nc.dram_tensor("wk", [HD, D], F32, kind="ExternalInput").ap()
    wv = nc.dram_tensor("wv", [HD, D], F32, kind="ExternalInput").ap()
    bq = nc.dram_tensor("bq", [HD], F32, kind="ExternalInput").ap()
    bk = nc.dram_tensor("bk", [HD], F32, kind="ExternalInput").ap()
    bv = nc.dram_tensor("bv", [HD], F32, kind="ExternalInput").ap()
    wo = nc.dram_tensor("wo", [D, HD], F32, kind="ExternalInput").ap()
    bo = nc.dram_tensor("bo", [D], F32, kind="ExternalInput").ap()

    # attention weights in TRANSPOSED layout: [head, key, query]
    attn_out = nc.dram_tensor("attn", [HPC, S, S], F32,
                              kind="ExternalOutput").ap()
    out_part = nc.dram_tensor("out", [S, D], F32, kind="ExternalOutput").ap()

    with tile.TileContext(nc) as tc:
        with ExitStack() as ctx:
            _body(ctx, tc, xq, xk, xv, wq, wk, wv, bq, bk, bv, wo, bo,
                  attn_out, out_part)
    nc.compile()
    return nc


def _body(ctx, tc, xq, xk, xv, wq, wk, wv, bq, bk, bv, wo, bo,
          attn_out, out_part):
    nc = tc.nc

    const_pool = ctx.enter_context(tc.tile_pool(name="const", bufs=1))
    wt_pool = ctx.enter_context(tc.tile_pool(name="wt", bufs=1))
    act_pool = ctx.enter_context(tc.tile_pool(name="acts", bufs=1))

    ident = const_pool.tile([P, P], F32, tag="ident")
    make_identity(nc, ident)

    # biases striped per-partition for transposed-layout outputs: [P, 2]
    bqv = const_pool.tile([P, 2], F32, tag="bqv")
    nc.sync.dma_start(bqv, bq.rearrange("(a p) -> p a", p=P))
    bkv = const_pool.tile([P, 2], F32, tag="bkv")
    nc.sync.dma_start(bkv, bk.rearrange("(a p) -> p a", p=P))
    # b_V / b_O broadcast across partitions (bias along the free axis)
    bv_row = const_pool.tile([1, HD], F32, tag="bvrow")
    nc.sync.dma_start(bv_row, bv[None, :])
    bvb = const_pool.tile([P, HD], F32, tag="bvb")
    nc.gpsimd.partition_broadcast(bvb, bv_row)
    bo_row = const_pool.tile([1, D], F32, tag="borow")
    nc.sync.dma_start(bo_row, bo[None, :])
    bob = const_pool.tile([P, D], F32, tag="bob")
    nc.gpsimd.partition_broadcast(bob, bo_row)

    # persistent activations
    qT = act_pool.tile([P, 2, S], F32, tag="qT")      # q^T  [256, S]
    kT = act_pool.tile([P, 2, S], F32, tag="kT")      # k^T  [256, S]
    # v natural [S, 4 heads x (64 dims + ones col)]
    vno = act_pool.tile([P, S // P, HPC * DK1], F32, tag="vno")
    outT = act_pool.tile([P, 2, S], F32, tag="outT")  # attn-out^T [256, S]

    # weights, transposed for use as matmul operands
    wqT = wt_pool.tile([P, 8, HD], F32, tag="wqT")    # W_Q[hs].T [1024, 256]
    wkT = wt_pool.tile([P, 8, HD], F32, tag="wkT")
    wvT = wt_pool.tile([P, 8, HD], F32, tag="wvT")
    woT = wt_pool.tile([P, 2, D], F32, tag="woT")     # W_O[:,hs].T [256, 1024]

    copy_engines = [
        lambda dst, src: nc.vector.tensor_copy(dst, src),
        lambda dst, src: nc.scalar.copy(dst, src),
    ]

    # ones columns of vno (positions h*65+64); v dims overwritten below.
    # memset can't target f32r directly (codegen rejects the value type),
    # so memset fp32 then round in place for the fp32r matmul consumers.
    ones_view = vno.rearrange("p m (h e) -> p m h e", e=DK1)[:, :, :, DK:]
    nc.vector.memset(ones_view, 1.0)
    nc.vector.tensor_copy(_r(ones_view), ones_view)

    # ---- phase 1+2: weight transposes, X^T, q/k/v projections ----
    with tc.tile_pool(name="stage", bufs=2) as stage_pool, \
         tc.tile_pool(name="xt", bufs=1) as xt_pool, \
         tc.tile_pool(name="tpsum", bufs=3, space="PSUM") as tpsum, \
         tc.tile_pool(name="mmpsum", bufs=2, space="PSUM") as mmpsum:

        cb_idx = [0]

        def pe_transpose4(srcs, dst):
            """Transpose four [128,128] blocks through one PSUM bank, then
            copy all four out in one batched engine copy. dst is
            [128, 4, 128] (possibly strided); written as float32r since
            these tiles feed fp32r matmuls."""
            pt = tpsum.tile([P, 512], F32, tag="tp")
            for j, s in enumerate(srcs):
                nc.tensor.transpose(pt[:, ts(j, P)], s, ident)
            copy_engines[cb_idx[0] % 2](
                _r(dst), pt.rearrange("p (a b) -> p a b", b=P))
            cb_idx[0] += 1

        # -- W_Q/K/V slices [256, 1024] -> transposed [1024, 256]
        for wdram, wT in ((wq, wqT), (wk, wkT), (wv, wvT)):
            wnat = stage_pool.tile([P, 2, D], F32, tag="wnat")
            nc.sync.dma_start(wnat, wdram.rearrange("(a p) f -> p a f", p=P))
            for a in range(2):
                for kg in range(2):
                    pe_transpose4(
                        [wnat[:, a, ts(kg * 4 + j, P)] for j in range(4)],
                        wT[:, kg * 4:(kg + 1) * 4, ts(a, P)])
        # -- W_O slice [1024, 256] -> transposed [256, 1024]
        wonat = stage_pool.tile([P, 8, HD], F32, tag="wnat")
        nc.sync.dma_start(wonat, wo.rearrange("(c p) f -> p c f", p=P))
        for a in range(2):
            for cg in range(2):
                pe_transpose4(
                    [wonat[:, cg * 4 + j, ts(a, P)] for j in range(4)],
                    woT[:, a, ts(cg, 512)].rearrange("p (a b) -> p a b", b=P))

        # -- X^T (per input, per half-sequence) + projections
        for t_idx, xdram in enumerate((xq, xk, xv)):
            for sh in range(2):     # sequence halves of 1024 tokens
                xT = xt_pool.tile([P, 8, 1024], F32, tag="xT")
                for st in range(8):
                    xnat = stage_pool.tile([P, D], F32, tag="xnat")
                    nc.sync.dma_start(
                        xnat, xdram[sh * 1024 + st * P: sh * 1024 + (st + 1) * P, :])
                    for kg in range(2):
                        pe_transpose4(
                            [xnat[:, ts(kg * 4 + j, P)] for j in range(4)],
                            xT[:, kg * 4:(kg + 1) * 4, ts(st, P)])
                if t_idx < 2:
                    # q/k in transposed layout: [256, S]
                    wT = (wqT, wkT)[t_idx]
                    bias = (bqv, bkv)[t_idx]
                    dst = (qT, kT)[t_idx]
                    for a in range(2):
                        for n in range(2):   # 512-token chunks in this half
                            ps = mmpsum.tile([P, 512], F32, tag="mm")
                            for kb in range(8):
                                nc.tensor.matmul(
                                    ps, _r(wT[:, kb, ts(a, P)]),
                                    _r(xT[:, kb, ts(n, 512)]),
                                    start=(kb == 0), stop=(kb == 7))
                            nc.vector.tensor_scalar_add(
                                _r(dst[:, a, sh * 1024 + n * 512:
                                       sh * 1024 + (n + 1) * 512]),
                                ps, bias[:, a:a + 1])
                else:
                    # v natural layout [S, 4x(64+1)]; ones columns untouched
                    for m in range(8):   # 128-token tiles in this half
                        ps = mmpsum.tile([P, HD], F32, tag="mm")
                        for kb in range(8):
                            nc.tensor.matmul(
                                ps[:, :HD], _r(xT[:, kb, ts(m, P)]),
                                _r(wvT[:, kb, :]),
                                start=(kb == 0), stop=(kb == 7))
                        vslice = vno[:, sh * 8 + m, :].rearrange(
                            "p (h e) -> p h e", e=DK1)[:, :, :DK]
                        nc.vector.tensor_add(
                            _r(vslice),
                            ps[:, :HD].rearrange("p (h e) -> p h e", e=DK),
                            bvb.rearrange("p (h e) -> p h e", e=DK))

    # ---- phase 3: attention (scoresT orientation; no transposes) ----
    n_qc = S // 512               # query chunks of 512
    with tc.tile_pool(name="expT", bufs=1) as expT_pool, \
         tc.tile_pool(name="attnT", bufs=4) as attnT_pool, \
         tc.tile_pool(name="rec", bufs=4) as rec_pool, \
         tc.tile_pool(name="scpsum", bufs=4, space="PSUM") as scpsum, \
         tc.tile_pool(name="avpsum", bufs=1, space="PSUM") as avpsum:

        for hp in range(2):          # head pairs (2*hp, 2*hp+1)
            for qc in range(n_qc):
                eT = [expT_pool.tile([P, 16, 512], F32, tag=f"expT{j}",
                                     name=f"expT{j}")
                      for j in range(2)]
                # scoresT tiles + exp, heads interleaved (disjoint PE rows)
                for kt in range(16):
                    for h01 in range(2):
                        po = 64 * h01
                        sps = scpsum.tile([P, 512], F32, tag="sc")
                        nc.tensor.matmul(
                            sps, _r(kT[po:po + 64, hp, ts(kt, P)]),
                            _r(qT[po:po + 64, hp, ts(qc, 512)]),
                            start=True, stop=True)
                        # exp(score/8) straight into the AV operand layout
                        nc.scalar.activation(
                            _r(eT[h01][:, kt, :]), sps, AFT.Exp,
                            bias=0.0, scale=0.125)
                for h01 in range(2):
                    h = 2 * hp + h01
                    # attn @ [v | 1]: row 64 = softmax denominator
                    avps = avpsum.tile([DK1, 512], F32, tag=f"av{h01}",
                                       name=f"av{h01}")
                    for kt in range(16):
                        nc.tensor.matmul(
                            avps, _r(vno[:, kt, ts(h, DK1)]),
                            _r(eT[h01][:, kt, :]),
                            start=(kt == 0), stop=(kt == 15))
                    recip = rec_pool.tile([1, 512], F32, tag="recip")
                    nc.vector.reciprocal(recip, avps[DK:DK1, :])
                    recb = rec_pool.tile([P, 512], F32, tag="recb")
                    nc.gpsimd.partition_broadcast(recb, recip)
                    # normalized attn-out rows -> outT
                    nc.vector.tensor_mul(
                        _r(outT[64 * h01:64 * (h01 + 1), hp, ts(qc, 512)]),
                        avps[:DK, :], recb[:DK, :])
                    # normalized attn^T tiles -> DRAM
                    for kt in range(16):
                        at = attnT_pool.tile([P, 512], F32, tag="at")
                        nc.vector.tensor_mul(at, eT[h01][:, kt, :], recb)
                        nc.sync.dma_start(
                            attn_out[h, ts(kt, P), ts(qc, 512)], at)

    # ---- phase 4: output projection ----
    with tc.tile_pool(name="ostage", bufs=3) as ostage, \
         tc.tile_pool(name="opsum", bufs=2, space="PSUM") as opsum:
        for mt in range(S // P):
            stg = ostage.tile([P, D], F32, tag="ostg")
            for n in range(2):
                ps = opsum.tile([P, 512], F32, tag="op")
                for a in range(2):
                    nc.tensor.matmul(
                        ps, _r(outT[:, a, ts(mt, P)]), _r(woT[:, a, ts(n, 512)]),
                        start=(a == 0), stop=(a == 1))
                nc.vector.tensor_add(stg[:, ts(n, 512)], ps, bob[:, ts(n, 512)])
            nc.sync.dma_start(out_part[ts(mt, P), :], stg)


_NC_CACHE = []


def _get_nc():
    if not _NC_CACHE:
        _NC_CACHE.append(build_nc())
    return _NC_CACHE[0]


def _shard_inputs(Q, K, V, W_Q, b_Q, W_K, b_K, W_V, b_V, W_O, b_O):
    in_maps = []
    for c in range(N_CORES):
        b = c // 4
        g = c % 4
        hs = slice(g * HD, (g + 1) * HD)
        in_maps.append({
            "xq": np.ascontiguousarray(Q[b]),
            "xk": np.ascontiguousarray(K[b]),
            "xv": np.ascontiguousarray(V[b]),
            "wq": np.ascontiguousarray(W_Q[hs]),
            "wk": np.ascontiguousarray(W_K[hs]),
            "wv": np.ascontiguousarray(W_V[hs]),
            "bq": np.ascontiguousarray(b_Q[hs]),
            "bk": np.ascontiguousarray(b_K[hs]),
            "bv": np.ascontiguousarray(b_V[hs]),
            "wo": np.ascontiguousarray(W_O[:, hs]),
            # b_O added by exactly one core per batch (partials are summed)
            "bo": np.ascontiguousarray(b_O) if g == 0 else np.zeros_like(b_O),
        })
    return in_maps


def run(inputs, trace=False):
    nc = _get_nc()
    in_maps = _shard_inputs(**inputs)
    res = run_bass_kernel_spmd(nc, in_maps, core_ids=list(range(N_CORES)),
                               trace=trace)
    output = np.zeros((B, S, D), np.float32)
    attn_w = np.zeros((B, H, S, S), np.float32)
    for c in range(N_CORES):
        b = c // 4
        g = c % 4
        output[b] += res.results[c]["out"]
        # device wrote attn^T [h, k, q]; transpose back to [h, q, k]
        attn_w[b, g * HPC:(g + 1) * HPC] = \
            res.results[c]["attn"].transpose(0, 2, 1)
    return (output, attn_w), res


def kernel(**inputs):
    (output, attn_w), _ = run(inputs, trace=False)
    return (output, attn_w)
